# revision 11
# baseline (speedup 1.0000x reference)
"""Multi-head causal attention (B=2, S=2048, D=1024, H=16) on 8 TRN2 cores.

Sharding: core c -> batch b = c//4, head group g = c%4 (4 heads each).
Per core, on device:
  - Q/K/V projections for its 4 heads (column-parallel W slices),
    producing Qt/Kt in [head_dim, seq] layout and V in [seq, head_dim].
  - Per head: k-major logits -> exp -> (ones-augmented V) matmul giving
    unnormalized attention + softmax denominator; then q-major logits ->
    exp with bias = -ln(denom) writing normalized weights straight to DRAM.
    Causal structure: strictly-upper blocks are never computed (they are
    exactly zero); diagonal blocks masked in-place with affine_select.
  - Row-parallel output projection -> partial out (host sums 4 partials
    per batch and adds bo).

Outputs: (out [B,S,D] f32, weights [B,H,S,S] f32) matching the reference.
"""

import math

import numpy as np
import ml_dtypes

import concourse.bass as bass
from concourse import bacc
import concourse.mybir as mybir
import concourse.tile as tile
from concourse.masks import make_identity

F32 = mybir.dt.float32
F32R = mybir.dt.float32r
BF16 = mybir.dt.bfloat16

P = 128  # SBUF partitions
QT = 512  # wide free-dim tile for logits


class Cfg:
    def __init__(self, B=2, S=2048, D=1024, H=16, ncores=8):
        self.B, self.S, self.D, self.H, self.ncores = B, S, D, H, ncores
        self.HD = D // H  # head dim (must be 64)
        assert self.HD == 64
        self.HC = H // (ncores // B)  # heads per core
        self.DC = self.HC * self.HD  # out-dims per core (q/k/v slice width)
        self.PT = max(1, self.DC // P)  # head-pair partition tiles of Qt/Kt
        assert self.DC % P == 0
        self.NDCH = D // P  # d-chunks for projections
        self.NQ5 = S // QT  # 512-wide tiles along seq
        self.NQ1 = S // P  # 128-tall tiles along seq
        self.SCALE = 1.0 / math.sqrt(self.HD)


CFG = Cfg()


def _blk(qt4, kt):
    """Flat index of causal k-major block (qt4: 512-q tile, kt: 128-k tile)."""
    return 2 * qt4 * (qt4 + 1) + kt


def build_bass(cfg: Cfg):
    c = cfg
    nc = bacc.Bacc()

    # Inputs (bf16 activations/weights, f32 biases). All are per-core shards.
    xqT = nc.declare_dram_parameter("xqT", [c.D, c.S], BF16, isOutput=False)
    xkT = nc.declare_dram_parameter("xkT", [c.D, c.S], BF16, isOutput=False)
    xvT = nc.declare_dram_parameter("xvT", [c.D + 1, c.S], BF16, isOutput=False)
    wqT = nc.declare_dram_parameter("wqT", [c.D, c.DC], BF16, isOutput=False)
    wkT = nc.declare_dram_parameter("wkT", [c.D, c.DC], BF16, isOutput=False)
    wvT = nc.declare_dram_parameter("wvT", [c.D + 1, c.DC], BF16, isOutput=False)
    woT = nc.declare_dram_parameter("woT", [c.DC, c.D], BF16, isOutput=False)
    bqp = nc.declare_dram_parameter("bqp", [P, c.PT], F32, isOutput=False)
    bkp = nc.declare_dram_parameter("bkp", [P, c.PT], F32, isOutput=False)

    w_out = nc.declare_dram_parameter("w_out", [c.HC, c.S, c.S], F32, isOutput=True)
    out_p = nc.declare_dram_parameter("out_p", [c.S, c.D], F32, isOutput=True)

    xqT_r = xqT.rearrange("(ch p) s -> p ch s", p=P)
    xkT_r = xkT.rearrange("(ch p) s -> p ch s", p=P)
    xvT_r = xvT[: c.D, :].rearrange("(ch p) s -> p ch s", p=P)
    NBLK = _blk(c.NQ5 - 1, 4 * (c.NQ5 - 1) + 4)  # total causal blocks

    with tile.TileContext(nc) as tc:
        with (
            tc.tile_pool(name="consts", bufs=1) as consts,
            tc.tile_pool(name="persist", bufs=1) as persist,
            tc.tile_pool(name="xs", bufs=3) as xs,
            tc.tile_pool(name="expl", bufs=4) as expl_pool,
            tc.tile_pool(name="den", bufs=2) as den_pool,
            tc.tile_pool(name="wrow", bufs=3) as wrow_pool,
            tc.tile_pool(name="osb", bufs=2) as osb_pool,
            tc.tile_pool(name="ps_l", bufs=2, space="PSUM") as ps_l,
            tc.tile_pool(name="ps_att", bufs=1, space="PSUM") as ps_att,
        ):
            # ---- constants ----
            wq_sb = consts.tile([P, c.NDCH, c.DC], BF16, tag="wq")
            wk_sb = consts.tile([P, c.NDCH, c.DC], BF16, tag="wk")
            wv_sb = consts.tile([P, c.NDCH, c.DC], BF16, tag="wv")
            wvb_sb = consts.tile([1, c.DC], BF16, tag="wvb")
            wo_sb = consts.tile([P, c.DC // P, c.D], BF16, tag="wo")
            bq_sb = consts.tile([P, c.PT], F32, tag="bq")
            bk_sb = consts.tile([P, c.PT], F32, tag="bk")
            ident = consts.tile([P, P], BF16, tag="ident")
            zrow = consts.tile([1, QT], BF16, tag="zrow")

            nc.sync.dma_start(out=wq_sb, in_=wqT.rearrange("(ch p) m -> p ch m", p=P))
            nc.sync.dma_start(out=wk_sb, in_=wkT.rearrange("(ch p) m -> p ch m", p=P))
            nc.sync.dma_start(
                out=wv_sb, in_=wvT[: c.D, :].rearrange("(ch p) m -> p ch m", p=P)
            )
            nc.gpsimd.dma_start(out=wvb_sb, in_=wvT[c.D : c.D + 1, :])
            nc.sync.dma_start(out=wo_sb, in_=woT.rearrange("(ch p) n -> p ch n", p=P))
            nc.gpsimd.dma_start(out=bq_sb, in_=bqp[:, :])
            nc.gpsimd.dma_start(out=bk_sb, in_=bkp[:, :])
            make_identity(nc, ident)
            nc.vector.memset(zrow, 0.0)

            # ---- persistent activations ----
            qt_sb = persist.tile([P, c.PT, c.S], F32R, tag="qt")  # [hd(2 heads), S]
            kt_sb = persist.tile([P, c.PT, c.S], F32R, tag="kt")
            v_sb = persist.tile([P, c.NQ1, c.HC, 66], BF16, tag="v")  # [s, kt, h, hd+1(+pad)]
            attn_sb = persist.tile([P, c.NQ1, c.HC, c.HD], BF16, tag="attn")

            nc.vector.memset(v_sb[:, :, :, 64:65], 1.0)  # ones col for denominator

            # ---- P1: projections ----
            # Qt/Kt: out[hd_pair, seq] = W_slice @ X^T
            for src, wsb, bsb, dst in (
                (xqT_r, wq_sb, bq_sb, qt_sb),
                (xkT_r, wk_sb, bk_sb, kt_sb),
            ):
                for qt in range(c.NQ5):
                    x_t = xs.tile([P, c.NDCH, QT], BF16, tag="x")
                    nc.sync.dma_start(out=x_t, in_=src[:, :, qt * QT : (qt + 1) * QT])
                    for pt in range(c.PT):
                        ps = ps_l.tile([P, QT], F32, tag="l")
                        for ch in range(c.NDCH):
                            nc.tensor.matmul(
                                ps,
                                wsb[:, ch, pt * P : (pt + 1) * P],
                                x_t[:, ch, :],
                                start=(ch == 0),
                                stop=(ch == c.NDCH - 1),
                            )
                        nc.vector.tensor_scalar_add(
                            out=dst[:, pt, qt * QT : (qt + 1) * QT],
                            in0=ps,
                            scalar1=bsb[:, pt : pt + 1],
                        )
            # V: out[seq, hd] = X @ Wv_slice^T + bv (bias folded via ones row)
            for st in range(c.NQ1):
                xv_t = xs.tile([P, c.NDCH, P], BF16, tag="x")
                xvb_t = xs.tile([1, P], BF16, tag="xb")
                nc.sync.dma_start(out=xv_t, in_=xvT_r[:, :, st * P : (st + 1) * P])
                nc.sync.dma_start(
                    out=xvb_t, in_=xvT[c.D : c.D + 1, st * P : (st + 1) * P]
                )
                ps = ps_l.tile([P, c.DC], F32, tag="l")
                for ch in range(c.NDCH):
                    nc.tensor.matmul(
                        ps, xv_t[:, ch, :], wv_sb[:, ch, :],
                        start=(ch == 0), stop=False,
                    )
                nc.tensor.matmul(ps, xvb_t, wvb_sb, start=False, stop=True)
                nc.vector.tensor_copy(
                    out=v_sb[:, st, :, 0:64],
                    in_=ps.rearrange("p (h d) -> p h d", h=c.HC),
                )

            # ---- P2: attention per head ----
            # Fused k-outer loop: for each 128-row k tile, exp(logits) over
            # its valid (causal) q window, then immediately accumulate
            # att[q1] += expl_slice @ V_tile in a psum-resident accumulator.
            # Keeps the PE dense (no phase-global ACT barrier per head).
            LW = min(2 * QT, c.S)  # logits psum chunk (<= 2 banks)
            for h in range(c.HC):
                hp, po = h // 2, (h % 2) * 64
                qs = qt_sb[po : po + 64, hp, :]  # [64, S] f32r
                ks = kt_sb[po : po + 64, hp, :]

                att_ps = ps_att.tile([P, c.NQ1, P], F32, tag="att")  # cols 0:65 used
                # open each psum bank's accumulation group with a zeroing
                # K=1 matmul (start=True clears has_written bank-wide; per-q1
                # start flags would clobber sibling slices in the same bank)
                for bk4 in range(c.NQ1 // 4):
                    nc.tensor.matmul(
                        att_ps[:, 4 * bk4 : 4 * bk4 + 4, :].rearrange(
                            "p a b -> p (a b)"
                        ),
                        zrow[:, 0:P], zrow[:, 0:QT],
                        start=True, stop=False,
                    )
                for kt in range(c.NQ1):
                    qw0 = QT * (kt // 4)  # causal q-window start (block granular)
                    W = c.S - qw0
                    ex_t = expl_pool.tile([P, c.S], BF16, tag="expl")
                    for off in range(0, W, LW):
                        cw = min(LW, W - off)
                        ps = ps_l.tile([P, LW], F32, tag="l")
                        for o2 in range(0, cw, QT):
                            nc.tensor.matmul(
                                ps[:, o2 : o2 + QT],
                                ks[:, kt * P : (kt + 1) * P],
                                qs[:, qw0 + off + o2 : qw0 + off + o2 + QT],
                                start=True, stop=True,
                            )
                        nc.scalar.activation(
                            out=ex_t[:, off : off + cw], in_=ps[:, :cw],
                            func=mybir.ActivationFunctionType.Exp,
                            scale=c.SCALE,
                        )
                    # causal mask of the diagonal 512-block (window start)
                    nc.gpsimd.affine_select(
                        out=ex_t[:, 0:QT], in_=ex_t[:, 0:QT],
                        compare_op=mybir.AluOpType.is_ge,
                        fill=0.0, base=qw0 - P * kt,
                        pattern=[[1, QT]], channel_multiplier=-1,
                    )
                    for q1 in range(kt, c.NQ1):
                        nc.tensor.matmul(
                            att_ps[:, q1, 0:65],
                            ex_t[:, q1 * P - qw0 : (q1 + 1) * P - qw0],
                            v_sb[:, kt, h, 0:65],
                            start=False,
                            stop=(kt == q1 and kt % 4 == 3),
                        )

                rden = den_pool.tile([P, c.NQ1], F32, tag="rden")
                negln = den_pool.tile([P, c.NQ1], F32, tag="negln")
                nc.vector.reciprocal(
                    out=rden,
                    in_=att_ps[:, :, 64:65].rearrange("p a b -> p (a b)"),
                )
                for q1 in range(c.NQ1):
                    nc.vector.tensor_scalar_mul(
                        out=attn_sb[:, q1, h, :],
                        in0=att_ps[:, q1, 0:64],
                        scalar1=rden[:, q1 : q1 + 1],
                    )
                nc.scalar.activation(
                    out=negln, in_=rden, func=mybir.ActivationFunctionType.Ln
                )

                # Stage C: q-major normalized weights -> DRAM
                for q1 in range(c.NQ1):
                    nk4 = q1 // 4 + 1
                    wr = wrow_pool.tile([P, c.S], F32, tag="wr")
                    for off in range(0, nk4 * QT, LW):
                        cw = min(LW, nk4 * QT - off)
                        ps = ps_l.tile([P, LW], F32, tag="l")
                        for o2 in range(0, cw, QT):
                            nc.tensor.matmul(
                                ps[:, o2 : o2 + QT],
                                qs[:, q1 * P : (q1 + 1) * P],
                                ks[:, off + o2 : off + o2 + QT],
                                start=True, stop=True,
                            )
                        nc.scalar.activation(
                            out=wr[:, off : off + cw], in_=ps[:, :cw],
                            func=mybir.ActivationFunctionType.Exp,
                            scale=c.SCALE,
                            bias=negln[:, q1 : q1 + 1],
                        )
                    nc.gpsimd.affine_select(  # diagonal block
                        out=wr[:, (nk4 - 1) * QT : nk4 * QT],
                        in_=wr[:, (nk4 - 1) * QT : nk4 * QT],
                        compare_op=mybir.AluOpType.is_ge,
                        fill=0.0, base=P * q1 - QT * (nk4 - 1),
                        pattern=[[-1, QT]], channel_multiplier=1,
                    )
                    nc.sync.dma_start(
                        out=w_out[h, q1 * P : (q1 + 1) * P, 0 : QT * nk4],
                        in_=wr[:, 0 : QT * nk4],
                    )

            # ---- P3: transpose attention, output projection ----
            attT = persist.tile([P, c.DC // P, c.NQ1, P], BF16, tag="attT")
            for st in range(c.NQ1):
                for ch in range(c.DC // P):
                    ps = ps_att.tile([P, P], BF16, tag="att")
                    nc.tensor.transpose(
                        ps,
                        attn_sb[:, st, 2 * ch : 2 * ch + 2, :].rearrange(
                            "p a b -> p (a b)"
                        ),
                        ident,
                    )
                    nc.vector.tensor_copy(out=attT[:, ch, st, :], in_=ps)
            OT = min(QT, c.D)
            for st in range(c.NQ1):
                o_t = osb_pool.tile([P, c.D], F32, tag="ot")
                for nt in range(c.D // OT):
                    ps = ps_l.tile([P, OT], F32, tag="l")
                    for ch in range(c.DC // P):
                        nc.tensor.matmul(
                            ps,
                            attT[:, ch, st, :],
                            wo_sb[:, ch, nt * OT : (nt + 1) * OT],
                            start=(ch == 0), stop=(ch == c.DC // P - 1),
                        )
                    nc.vector.tensor_copy(out=o_t[:, nt * OT : (nt + 1) * OT], in_=ps)
                nc.sync.dma_start(
                    out=out_p[st * P : (st + 1) * P, :], in_=o_t
                )

    return nc


_NC_CACHE = {}


def _get_nc(cfg=CFG):
    key = (cfg.B, cfg.S, cfg.D, cfg.H, cfg.ncores)
    if key not in _NC_CACHE:
        nc = build_bass(cfg)
        if not nc.is_finalized():
            nc.finalize()
        _NC_CACHE[key] = nc
    return _NC_CACHE[key]


def make_core_inputs(cfg, query, key, value, Wq, bq, Wk, bk, Wv, bv, Wo, bo):
    """Host-side sharding: returns per-core input maps."""
    c = cfg
    bf = ml_dtypes.bfloat16
    per_batch = []
    for b in range(c.B):
        xqT = np.ascontiguousarray(query[b].T).astype(bf)
        xkT = np.ascontiguousarray(key[b].T).astype(bf)
        xvT = np.concatenate(
            [value[b].T, np.ones((1, c.S), np.float32)], axis=0
        ).astype(bf)
        per_batch.append((xqT, xkT, xvT))
    WqT = np.ascontiguousarray(Wq.T)  # [D, D] in-dim major
    WkT = np.ascontiguousarray(Wk.T)
    WvT = np.concatenate([Wv.T, bv[None, :]], axis=0)  # [D+1, D]
    WoT = np.ascontiguousarray(Wo.T)  # [D(in), D(out)]

    in_maps = []
    for core in range(c.ncores):
        b, g = core // (c.ncores // c.B), core % (c.ncores // c.B)
        lo, hi = g * c.DC, (g + 1) * c.DC
        xqT, xkT, xvT = per_batch[b]
        in_maps.append(
            {
                "xqT": xqT,
                "xkT": xkT,
                "xvT": xvT,
                "wqT": WqT[:, lo:hi].astype(bf),
                "wkT": WkT[:, lo:hi].astype(bf),
                "wvT": WvT[:, lo:hi].astype(bf),
                "woT": np.ascontiguousarray(WoT[lo:hi, :]).astype(bf),
                "bqp": np.ascontiguousarray(
                    bq[lo:hi].reshape(c.PT, P).T
                ).astype(np.float32),
                "bkp": np.ascontiguousarray(
                    bk[lo:hi].reshape(c.PT, P).T
                ).astype(np.float32),
            }
        )
    return in_maps


def assemble_outputs(cfg, results, bo):
    c = cfg
    out = np.zeros((c.B, c.S, c.D), np.float32)
    weights = np.zeros((c.B, c.H, c.S, c.S), np.float32)
    for core in range(c.ncores):
        b, g = core // (c.ncores // c.B), core % (c.ncores // c.B)
        out[b] += results[core]["out_p"]
        weights[b, g * c.HC : (g + 1) * c.HC] = results[core]["w_out"]
    out += bo[None, None, :]
    return out, weights


def _np_reference(query, key, value, mask, Wq, bq, Wk, bk, Wv, bv, Wo, bo):
    """Exact-semantics numpy fallback (only used if mask is not causal)."""
    B_, S_, D_ = query.shape
    H_ = 16
    HD_ = D_ // H_

    def split(x):
        return x.reshape(B_, S_, H_, HD_).transpose(0, 2, 1, 3).reshape(B_ * H_, S_, HD_)

    q = split(query @ Wq.T + bq)
    k = split(key @ Wk.T + bk)
    v = split(value @ Wv.T + bv)
    logits = np.einsum("bqd,bkd->bqk", q, k) / math.sqrt(HD_)
    logits = np.where(mask, logits, np.finfo(np.float32).min)
    logits = logits - logits.max(axis=-1, keepdims=True)
    w = np.exp(logits)
    w /= w.sum(axis=-1, keepdims=True)
    att = np.einsum("bqk,bkd->bqd", w, v)
    att = att.reshape(B_, H_, S_, HD_).transpose(0, 2, 1, 3).reshape(B_, S_, D_)
    out = att @ Wo.T + bo
    return out.astype(np.float32), w.reshape(B_, H_, S_, S_).astype(np.float32)


def kernel(query, key, value, mask, Wq, bq, Wk, bk, Wv, bv, Wo, bo):
    args = [
        np.asarray(a, dtype=np.float32) if np.asarray(a).dtype != bool else np.asarray(a)
        for a in (query, key, value, mask, Wq, bq, Wk, bk, Wv, bv, Wo, bo)
    ]
    query, key, value, mask, Wq, bq, Wk, bk, Wv, bv, Wo, bo = args

    causal = np.array_equal(
        np.asarray(mask, dtype=bool), np.tril(np.ones(mask.shape, dtype=bool))
    )
    if not causal:
        return _np_reference(query, key, value, mask, Wq, bq, Wk, bk, Wv, bv, Wo, bo)

    from concourse.bass_utils import run_bass_kernel_spmd

    cfg = CFG
    nc = _get_nc(cfg)
    in_maps = make_core_inputs(cfg, query, key, value, Wq, bq, Wk, bk, Wv, bv, Wo, bo)
    res = run_bass_kernel_spmd(nc, in_maps, list(range(cfg.ncores))).results
    return assemble_outputs(cfg, res, bo)


if __name__ == "__main__":
    import reference

    inputs = {k: np.asarray(v) for k, v in reference.setup_inputs().items()}
    out, w = kernel(**inputs)
    print("out", out.shape, "weights", w.shape)
